# revision 33
# baseline (speedup 1.0000x reference)
"""InternLM3 self-attention (prefill, GQA, RoPE) on 8 Trainium2 cores.

Tensor-parallel over heads: core r owns q heads 4r..4r+3 and kv head r
(wqkv column shards, wo row shards).  Each core computes its partial
output projection; the 8 partials are summed on the host.

v3 design:
  - all matmuls bf16 (Fast Weight Load on; fp32r would serialize
    LDWEIGHTS at ~125 ns/matmul).
  - one software pipeline over the 4 token chunks: proj(t) -> rope(t)
    -> attention(g=t) -> out-proj(t); per-tile semaphores overlap the
    phases across engines.
  - softmax denominator accumulated on PE with an all-ones [128,128]
    stationary (result replicated across partitions), so 1/d is a single
    reciprocal_approx_fast on [128,512] and the normalizing multiply
    needs no broadcast at all.
  - scores/pv/denominator matmuls run 2 chunks behind the scores stream
    so PSUM bank recycling at head boundaries never stalls the PE.
  - RoPE rotate-half via SBUF->SBUF DMA partition shift; k head is
    roped first so attention never waits on it.
  - q and k both carry 128^-0.5 in the rope tables; exp() applies the
    compensating sqrt(128) via its free affine scale.
  - batched weight DMAs (descriptors fan out across all 16 queues);
    wo load is emitted late so it never delays the first projection.
"""

import numpy as np
import ml_dtypes

import concourse.bass as bass
import concourse.bacc as bacc
import concourse.mybir as mybir
import concourse.tile as tile
from concourse.bass_utils import run_bass_kernel_spmd

T = 2048
H = 4096
NH = 32
NKV = 8
HD = 128
HALF = HD // 2
BASE = 1000000.0
NCORES = 8
QH = NH // NCORES            # 4 q heads per core
QCOLS = QH * HD              # 512
SH_COLS = QCOLS + 2 * HD     # 768 wqkv cols per core
NEG = -1e30

P = 128
TC = 512                     # token chunk
NT = T // TC                 # 4
NHC = H // P                 # 32 contraction chunks
NQC = SH_COLS // P           # 6 qkv col chunks (0-3 q heads, 4 k, 5 v)
NOC = H // TC                # 8 output col chunks
LAG = 3                      # chunks the pv/denominator stream trails by

f32 = mybir.dt.float32
bf16 = mybir.dt.bfloat16

EXP_SCALE = float(np.sqrt(128.0))   # tables carry 128^-0.5 on q and k

_COMPILED = None


def _build():
    nc = bacc.Bacc("TRN2", target_bir_lowering=False, debug=False,
                   num_devices=NCORES)

    hidT = nc.dram_tensor("hidT", [H, T], bf16, kind="ExternalInput").ap()
    wqkv_s = nc.dram_tensor("wqkv_s", [H, SH_COLS], bf16,
                            kind="ExternalInput").ap()
    wo_s = nc.dram_tensor("wo_s", [QCOLS, H], bf16,
                          kind="ExternalInput").ap()
    cosq = nc.dram_tensor("cosq", [P, T], bf16, kind="ExternalInput").ap()
    sinq = nc.dram_tensor("sinq", [P, T], bf16, kind="ExternalInput").ap()
    masks = nc.dram_tensor("masks", [P, P], bf16,
                           kind="ExternalInput").ap()
    ident = nc.dram_tensor("ident", [P, P], bf16, kind="ExternalInput").ap()
    onesm = nc.dram_tensor("onesm", [P, P], bf16, kind="ExternalInput").ap()
    rperm = nc.dram_tensor("rperm", [P, P], bf16, kind="ExternalInput").ap()
    part = nc.dram_tensor("part", [T, H], bf16, kind="ExternalOutput").ap()

    with tile.TileContext(nc) as tc:
        with tc.tile_pool(name="keep", bufs=1) as keep, \
             tc.tile_pool(name="hid", bufs=10) as hidp, \
             tc.tile_pool(name="attn", bufs=2) as attp, \
             tc.tile_pool(name="rot", bufs=2) as rotp, \
             tc.tile_pool(name="e", bufs=6) as ep, \
             tc.tile_pool(name="es", bufs=5) as esp, \
             tc.tile_pool(name="rbs", bufs=1) as rbsp, \
             tc.tile_pool(name="ob", bufs=4) as obp, \
             tc.tile_pool(name="ps", bufs=6, space="PSUM") as psp, \
             tc.tile_pool(name="acc", bufs=2, space="PSUM") as accp:

            # ---- long-lived SBUF ----
            wq = keep.tile([P, NHC, SH_COLS], bf16)        # 48 KB/part
            wo_r = keep.tile([P, QH, H], bf16)             # 32 KB/part
            qkvT = keep.tile([P, NQC, T], bf16)            # 24 KB/part
            ct = keep.tile([P, T], bf16, tag="cosq_t")     # 4 KB
            st_t = keep.tile([P, T], bf16, tag="sinq_t")   # 4 KB
            mt = keep.tile([P, P], bf16, tag="masks_t")    # staircase
            vnat = keep.tile([P, T // P, P], bf16, tag="vnat")  # 4 KB
            idt = keep.tile([P, P], bf16, tag="ident_t")
            o_m = keep.tile([P, P], bf16, tag="onesm_t")
            rp = keep.tile([P, P], bf16, tag="rperm_t")

            # hid(0) + wq interleaved in first-needed order: the DMA queues
            # are FIFO, so issue order decides who gets early bandwidth.
            QTR = NHC // 8
            hid_t0 = [hidp.tile([P, QTR, TC], bf16, tag="hid",
                                name=f"hid_0_{qi}") for qi in range(8)]

            def hid_dma(hq, qi, t):
                nc.sync.dma_start(
                    hq[:],
                    hidT[qi * QTR * P:(qi + 1) * QTR * P,
                         t * TC:(t + 1) * TC].rearrange(
                        "(h p) n -> p h n", p=P))

            def wq_dma(c, half):
                hh = half * (NHC // 2)
                nc.sync.dma_start(
                    wq[:, hh:hh + NHC // 2, c * P:(c + 1) * P],
                    wqkv_s[hh * P:hh * P + NHC // 2 * P,
                           c * P:(c + 1) * P].rearrange(
                        "(h p) c -> p h c", p=P))

            def wq_dma_q(c, q8):
                hh = q8 * (NHC // 4)
                nc.sync.dma_start(
                    wq[:, hh:hh + NHC // 4, c * P:(c + 1) * P],
                    wqkv_s[hh * P:hh * P + NHC // 4 * P,
                           c * P:(c + 1) * P].rearrange(
                        "(h p) c -> p h c", p=P))

            hid_dma(hid_t0[0], 0, 0)
            wq_dma_q(4, 0)
            hid_dma(hid_t0[1], 1, 0)
            wq_dma_q(4, 1)
            hid_dma(hid_t0[2], 2, 0)
            wq_dma_q(4, 2)
            hid_dma(hid_t0[3], 3, 0)
            wq_dma_q(4, 3)
            for qi in range(4, 8):
                hid_dma(hid_t0[qi], qi, 0)
            for c in (5, 0, 1, 2, 3):
                wq_dma(c, 0)
                wq_dma(c, 1)
            nc.scalar.dma_start(ct[:], cosq[:])
            nc.scalar.dma_start(st_t[:], sinq[:])
            nc.scalar.dma_start(mt[:], masks[:])
            nc.scalar.dma_start(idt[:], ident[:])
            nc.scalar.dma_start(o_m[:], onesm[:])
            nc.scalar.dma_start(rp[:], rperm[:])

            op_pending = []
            for t in range(NT):
                tsl = slice(t * TC, (t + 1) * TC)

                if t == 0:
                    hid_q = hid_t0
                else:
                    hid_q = []
                    for qi in range(8):
                        hq = hidp.tile([P, QTR, TC], bf16, tag="hid",
                                       name=f"hid_{t}_{qi}")
                        hid_dma(hq, qi, t)
                        hid_q.append(hq)

                # ---- phase 1: qkv^T chunk = wqkv^T @ hidden^T ----
                # k and v first so rope(k) / v-transpose overlap the q cols.
                def proj_col(c):
                    qps = psp.tile([P, TC], f32, tag="ps",
                                   name=f"qps_{t}_{c}")
                    for h in range(NHC):
                        nc.tensor.matmul(
                            qps[:], wq[:, h, c * P:(c + 1) * P],
                            hid_q[h // QTR][:, h % QTR, :],
                            start=(h == 0), stop=(h == NHC - 1))
                    nc.scalar.copy(qkvT[:, c, tsl], qps[:])

                def rope_col(idx):
                    x = qkvT[:, idx, tsl]
                    rot_ps = psp.tile([P, TC], f32, tag="ps",
                                      name=f"rotp_{t}_{idx}")
                    nc.tensor.matmul(rot_ps[:], rp[:], x,
                                     start=True, stop=True)
                    rot = rotp.tile([P, TC], bf16, tag="rot",
                                    name=f"rot_{t}_{idx}")
                    nc.vector.tensor_tensor(
                        rot[:], rot_ps[:], st_t[:, tsl],
                        mybir.AluOpType.mult)
                    nc.vector.tensor_tensor(
                        x, x, ct[:, tsl], mybir.AluOpType.mult)
                    nc.vector.tensor_tensor(
                        x, x, rot[:], mybir.AluOpType.add)

                # rope/vt are emitted one c-group late so the PSUM->SBUF
                # copy they read never stalls the in-order PE stream.
                proj_col(4)                      # k
                proj_col(5)                      # v
                rope_col(QH)                     # k rope (c4 copy now done)
                proj_col(0)
                for j in range(TC // P):         # v transpose
                    kc = t * (TC // P) + j
                    tp = psp.tile([P, TC], bf16, tag="ps", name=f"vt_{kc}")
                    nc.tensor.transpose(
                        tp[:, 0:P], qkvT[:, 5, kc * P:(kc + 1) * P], idt[:])
                    nc.scalar.copy(vnat[:, kc, :], tp[:, 0:P])
                proj_col(1)
                rope_col(0)
                proj_col(2)
                rope_col(1)
                proj_col(3)
                rope_col(2)

                if t == 0:
                    # wo load, deferred so it never races the hot path
                    for hc in range(QH):
                        nc.sync.dma_start(wo_r[:, hc, :],
                                          wo_s[hc * P:(hc + 1) * P, :])

                # ---- phase 4: causal attention, q group g == t ----
                # out-proj(t-1) matmul groups are interleaved into the
                # chunk stream: softmax bookkeeping paces DVE/ACT, so the
                # PE has slack here that the deferred projection fills.
                attnT = attp.tile([P, QH, TC], bf16, tag="attnT",
                                  name=f"attnT_{t}")
                kmax = 4 * (t + 1)
                ci = 0
                for head in range(QH):
                    if head == 1:
                        rope_col(3)              # q3 rope hidden under head 0
                    pv = accp.tile([P, TC], f32, tag="pv",
                                   name=f"pv_{t}_{head}")
                    es = []
                    pend = {}                    # binary-counter bf16 sums

                    def es_push(x, name):
                        lvl = 0
                        while lvl in pend:
                            prev = pend.pop(lvl)
                            s = esp.tile([P, TC], bf16, tag="es",
                                         name=f"{name}_l{lvl}")
                            nc.vector.tensor_tensor(
                                s[:], prev[:], x[:], mybir.AluOpType.add)
                            x = s
                            lvl += 1
                        pend[lvl] = x

                    def drain_one():
                        pkc, pe, plo = es.pop(0)
                        nc.tensor.matmul(pv[:, plo:], vnat[:, pkc, :],
                                         pe[:, plo:],
                                         start=(pkc == 0),
                                         stop=(pkc == kmax - 1))

                    for kc in range(kmax):
                        j = kc - 4 * t
                        lo = max(j, 0) * P   # cols < lo are fully masked
                        st_ps = psp.tile([P, TC], f32, tag="ps",
                                         name=f"st_{t}_{head}_{kc}")
                        nc.tensor.matmul(
                            st_ps[:, lo:],
                            qkvT[:, QH, kc * P:(kc + 1) * P],
                            qkvT[:, head, t * TC + lo:(t + 1) * TC],
                            start=True, stop=True)
                        if ci % 2 == 1 and op_pending:
                            op_pending.pop(0)()
                        ci += 1
                        if len(es) >= LAG:
                            drain_one()
                        e = ep.tile([P, TC], bf16, tag="E",
                                    name=f"e_{t}_{head}_{kc}")
                        if j >= 0:
                            # staircase lives in cols [128j, 128j+128)
                            nc.vector.tensor_tensor(
                                st_ps[:, lo:lo + P],
                                st_ps[:, lo:lo + P], mt[:],
                                mybir.AluOpType.add)
                            if j > 0:
                                nc.gpsimd.memset(e[:, 0:lo], 0)
                        nc.scalar.activation(
                            e[:, lo:], st_ps[:, lo:],
                            mybir.ActivationFunctionType.Exp,
                            scale=EXP_SCALE)
                        es.append((kc, e, lo))
                        es_push(e, f"es_{t}_{head}_{kc}")
                    while es:
                        drain_one()
                    root = None
                    for lvl in sorted(pend):
                        if root is None:
                            root = pend.pop(lvl)
                        else:
                            s = esp.tile([P, TC], bf16, tag="es",
                                         name=f"esr_{t}_{head}_{lvl}")
                            nc.vector.tensor_tensor(
                                s[:], root[:], pend.pop(lvl)[:],
                                mybir.AluOpType.add)
                            root = s
                    d_rep = psp.tile([P, TC], f32, tag="ps",
                                     name=f"d_{t}_{head}")
                    nc.tensor.matmul(d_rep[:], o_m[:], root[:],
                                     start=True, stop=True)
                    rbs = rbsp.tile([P, TC], f32, tag="rbs",
                                    name=f"rbs_{t}_{head}")
                    nc.vector.reciprocal_approx_fast(rbs[:], d_rep[:])
                    nc.vector.tensor_tensor(
                        attnT[:, head, :], pv[:], rbs[:],
                        mybir.AluOpType.mult)

                # any out-proj(t-1) groups that did not fit in the stream
                for g in op_pending:
                    g()

                # ---- phase 5: out chunk = attn(t) @ wo_shard, deferred ----
                def op_group(t_, oc, tcn, at):
                    def emit():
                        o_ps = psp.tile([P, TC], f32, tag="ps",
                                        name=f"o_{t_}_{oc}_{tcn}")
                        for hc in range(QH):
                            nc.tensor.matmul(
                                o_ps[:],
                                at[:, hc, tcn * P:(tcn + 1) * P],
                                wo_r[:, hc, oc * TC:(oc + 1) * TC],
                                start=(hc == 0), stop=(hc == QH - 1))
                        ob = obp.tile([P, TC], bf16, tag="ob",
                                      name=f"ob_{t_}_{oc}_{tcn}")
                        nc.scalar.copy(ob[:], o_ps[:])
                        nc.gpsimd.dma_start(
                            part[t_ * TC + tcn * P:t_ * TC + (tcn + 1) * P,
                                 oc * TC:(oc + 1) * TC], ob[:])
                    return emit

                op_pending = [op_group(t, oc, tcn, attnT)
                              for oc in range(NOC)
                              for tcn in range(TC // P)]

            # final token chunk's projection has no later phase to hide in
            for g in op_pending:
                g()

    nc.compile()
    return nc


def _rope_tables(positions):
    pos = positions.astype(np.float64)
    inv_freq = 1.0 / (BASE ** (np.arange(HALF, dtype=np.float64) / HALF))
    freqs = pos[:, None] * inv_freq[None, :]          # [T, 64]
    cos = np.cos(freqs)
    sin = np.sin(freqs)
    cosT = np.concatenate([cos, cos], axis=1).T       # [128, T]
    sinT = np.concatenate([-sin, sin], axis=1).T      # sign folded
    return cosT, sinT


def kernel(positions, hidden_states, wqkv, wo):
    global _COMPILED
    if _COMPILED is None:
        _COMPILED = _build()
    nc = _COMPILED

    s = 128.0 ** -0.5                                 # per-side score scale
    cosT, sinT = _rope_tables(positions)
    cosq = np.ascontiguousarray(cosT * s).astype(ml_dtypes.bfloat16)
    sinq = np.ascontiguousarray(sinT * s).astype(ml_dtypes.bfloat16)

    hidT = np.ascontiguousarray(hidden_states.T).astype(ml_dtypes.bfloat16)

    # causal staircase mask, ST layout [k, q]: one [128,128] tile serves
    # every diagonal block
    kl = np.arange(P)[:, None]
    ql = np.arange(P)[None, :]
    masks = np.where(kl <= ql, 0.0, NEG).astype(ml_dtypes.bfloat16)

    ident = np.eye(P, dtype=np.float32).astype(ml_dtypes.bfloat16)
    onesm = np.ones((P, P), dtype=np.float32).astype(ml_dtypes.bfloat16)
    rperm_np = np.zeros((P, P), dtype=np.float32)
    for m in range(P):
        rperm_np[(m + HALF) % P, m] = 1.0             # out[m]=x[(m+64)%128]
    rperm_np = rperm_np.astype(ml_dtypes.bfloat16)

    in_maps = []
    for r in range(NCORES):
        qc = slice(r * QCOLS, (r + 1) * QCOLS)
        kc = slice(NH * HD + r * HD, NH * HD + (r + 1) * HD)
        vc = slice((NH + NKV) * HD + r * HD, (NH + NKV) * HD + (r + 1) * HD)
        wqkv_s = np.ascontiguousarray(
            np.concatenate([wqkv[:, qc], wqkv[:, kc], wqkv[:, vc]],
                           axis=1)).astype(ml_dtypes.bfloat16)
        wo_s = np.ascontiguousarray(wo[qc, :]).astype(ml_dtypes.bfloat16)
        in_maps.append({
            "hidT": hidT, "wqkv_s": wqkv_s, "wo_s": wo_s,
            "cosq": cosq, "sinq": sinq, "masks": masks,
            "ident": ident, "onesm": onesm, "rperm": rperm_np,
        })

    global _LAST_IN_MAPS
    _LAST_IN_MAPS = in_maps
    res = run_bass_kernel_spmd(nc, in_maps, list(range(NCORES)))
    out = res.results[0]["part"].astype(np.float64)
    for r in range(1, NCORES):
        out += res.results[r]["part"].astype(np.float64)
    return out.astype(np.float32)


# revision 34
# speedup vs baseline: 1.0667x; 1.0667x over previous
"""InternLM3 self-attention (prefill, GQA, RoPE) on 8 Trainium2 cores.

Tensor-parallel over heads: core r owns q heads 4r..4r+3 and kv head r
(wqkv column shards, wo row shards).  Each core computes its partial
output projection; the 8 partials are summed on the host.

v3 design:
  - all matmuls bf16 (Fast Weight Load on; fp32r would serialize
    LDWEIGHTS at ~125 ns/matmul).
  - one software pipeline over the 4 token chunks: proj(t) -> rope(t)
    -> attention(g=t) -> out-proj(t); per-tile semaphores overlap the
    phases across engines.
  - softmax denominator accumulated on PE with an all-ones [128,128]
    stationary (result replicated across partitions), so 1/d is a single
    reciprocal_approx_fast on [128,512] and the normalizing multiply
    needs no broadcast at all.
  - scores/pv/denominator matmuls run 2 chunks behind the scores stream
    so PSUM bank recycling at head boundaries never stalls the PE.
  - RoPE rotate-half via SBUF->SBUF DMA partition shift; k head is
    roped first so attention never waits on it.
  - q and k both carry 128^-0.5 in the rope tables; exp() applies the
    compensating sqrt(128) via its free affine scale.
  - batched weight DMAs (descriptors fan out across all 16 queues);
    wo load is emitted late so it never delays the first projection.
"""

import numpy as np
import ml_dtypes

import concourse.bass as bass
import concourse.bacc as bacc
import concourse.mybir as mybir
import concourse.tile as tile
from concourse.bass_utils import run_bass_kernel_spmd

T = 2048
H = 4096
NH = 32
NKV = 8
HD = 128
HALF = HD // 2
BASE = 1000000.0
NCORES = 8
QH = NH // NCORES            # 4 q heads per core
QCOLS = QH * HD              # 512
SH_COLS = QCOLS + 2 * HD     # 768 wqkv cols per core
NEG = -1e30

P = 128
TC = 512                     # token chunk
NT = T // TC                 # 4
NHC = H // P                 # 32 contraction chunks
NQC = SH_COLS // P           # 6 qkv col chunks (0-3 q heads, 4 k, 5 v)
NOC = H // TC                # 8 output col chunks
LAG = 3                      # chunks the pv/denominator stream trails by

f32 = mybir.dt.float32
bf16 = mybir.dt.bfloat16

EXP_SCALE = float(np.sqrt(128.0))   # tables carry 128^-0.5 on q and k

_COMPILED = None


def _build():
    nc = bacc.Bacc("TRN2", target_bir_lowering=False, debug=False,
                   num_devices=NCORES)

    hidT = nc.dram_tensor("hidT", [H, T], bf16, kind="ExternalInput").ap()
    wqkv_s = nc.dram_tensor("wqkv_s", [H, SH_COLS], bf16,
                            kind="ExternalInput").ap()
    wo_s = nc.dram_tensor("wo_s", [QCOLS, H], bf16,
                          kind="ExternalInput").ap()
    cosq = nc.dram_tensor("cosq", [P, T], bf16, kind="ExternalInput").ap()
    sinq = nc.dram_tensor("sinq", [P, T], bf16, kind="ExternalInput").ap()
    masks = nc.dram_tensor("masks", [P, P], bf16,
                           kind="ExternalInput").ap()
    ident = nc.dram_tensor("ident", [P, P], bf16, kind="ExternalInput").ap()
    onesm = nc.dram_tensor("onesm", [P, P], bf16, kind="ExternalInput").ap()
    rperm = nc.dram_tensor("rperm", [P, P], bf16, kind="ExternalInput").ap()
    part = nc.dram_tensor("part", [T, H], bf16, kind="ExternalOutput").ap()

    with tile.TileContext(nc) as tc:
        with tc.tile_pool(name="keep", bufs=1) as keep, \
             tc.tile_pool(name="hid", bufs=10) as hidp, \
             tc.tile_pool(name="attn", bufs=2) as attp, \
             tc.tile_pool(name="rot", bufs=2) as rotp, \
             tc.tile_pool(name="e", bufs=7) as ep, \
             tc.tile_pool(name="rbs", bufs=2) as rbsp, \
             tc.tile_pool(name="ob", bufs=6) as obp, \
             tc.tile_pool(name="ps", bufs=6, space="PSUM") as psp, \
             tc.tile_pool(name="acc", bufs=1, space="PSUM") as accp:

            # ---- long-lived SBUF ----
            wq = keep.tile([P, NHC, SH_COLS], bf16)        # 48 KB/part
            wo_r = keep.tile([P, QH, H], bf16)             # 32 KB/part
            qkvT = keep.tile([P, NQC, T], bf16)            # 24 KB/part
            ct = keep.tile([P, T], bf16, tag="cosq_t")     # 4 KB
            st_t = keep.tile([P, T], bf16, tag="sinq_t")   # 4 KB
            mt = keep.tile([P, P], bf16, tag="masks_t")    # staircase
            vnat = keep.tile([P, T // P, P], bf16, tag="vnat")  # 4 KB
            idt = keep.tile([P, P], bf16, tag="ident_t")
            o_m = keep.tile([P, P], bf16, tag="onesm_t")
            rp = keep.tile([P, P], bf16, tag="rperm_t")

            # hid(0) + wq interleaved in first-needed order: the DMA queues
            # are FIFO, so issue order decides who gets early bandwidth.
            QTR = NHC // 8
            hid_t0 = [hidp.tile([P, QTR, TC], bf16, tag="hid",
                                name=f"hid_0_{qi}") for qi in range(8)]

            def hid_dma(hq, qi, t):
                nc.sync.dma_start(
                    hq[:],
                    hidT[qi * QTR * P:(qi + 1) * QTR * P,
                         t * TC:(t + 1) * TC].rearrange(
                        "(h p) n -> p h n", p=P))

            def wq_dma(c, half):
                hh = half * (NHC // 2)
                nc.sync.dma_start(
                    wq[:, hh:hh + NHC // 2, c * P:(c + 1) * P],
                    wqkv_s[hh * P:hh * P + NHC // 2 * P,
                           c * P:(c + 1) * P].rearrange(
                        "(h p) c -> p h c", p=P))

            def wq_dma_q(c, q8):
                hh = q8 * (NHC // 4)
                nc.sync.dma_start(
                    wq[:, hh:hh + NHC // 4, c * P:(c + 1) * P],
                    wqkv_s[hh * P:hh * P + NHC // 4 * P,
                           c * P:(c + 1) * P].rearrange(
                        "(h p) c -> p h c", p=P))

            hid_dma(hid_t0[0], 0, 0)
            wq_dma_q(4, 0)
            hid_dma(hid_t0[1], 1, 0)
            wq_dma_q(4, 1)
            hid_dma(hid_t0[2], 2, 0)
            wq_dma_q(4, 2)
            hid_dma(hid_t0[3], 3, 0)
            wq_dma_q(4, 3)
            for qi in range(4, 8):
                hid_dma(hid_t0[qi], qi, 0)
            for c in (5, 0, 1, 2, 3):
                wq_dma(c, 0)
                wq_dma(c, 1)
            nc.scalar.dma_start(ct[:], cosq[:])
            nc.scalar.dma_start(st_t[:], sinq[:])
            nc.scalar.dma_start(mt[:], masks[:])
            nc.scalar.dma_start(idt[:], ident[:])
            nc.scalar.dma_start(o_m[:], onesm[:])
            nc.scalar.dma_start(rp[:], rperm[:])

            for t in range(NT):
                tsl = slice(t * TC, (t + 1) * TC)

                if t == 0:
                    hid_q = hid_t0
                else:
                    hid_q = []
                    for qi in range(8):
                        hq = hidp.tile([P, QTR, TC], bf16, tag="hid",
                                       name=f"hid_{t}_{qi}")
                        hid_dma(hq, qi, t)
                        hid_q.append(hq)

                # ---- phase 1: qkv^T chunk = wqkv^T @ hidden^T ----
                # k and v first so rope(k) / v-transpose overlap the q cols.
                def proj_col(c):
                    qps = psp.tile([P, TC], f32, tag="ps",
                                   name=f"qps_{t}_{c}")
                    for h in range(NHC):
                        nc.tensor.matmul(
                            qps[:], wq[:, h, c * P:(c + 1) * P],
                            hid_q[h // QTR][:, h % QTR, :],
                            start=(h == 0), stop=(h == NHC - 1))
                    nc.scalar.copy(qkvT[:, c, tsl], qps[:])

                def rope_col(idx):
                    x = qkvT[:, idx, tsl]
                    rot_ps = psp.tile([P, TC], f32, tag="ps",
                                      name=f"rotp_{t}_{idx}")
                    nc.tensor.matmul(rot_ps[:], rp[:], x,
                                     start=True, stop=True)
                    rot = rotp.tile([P, TC], bf16, tag="rot",
                                    name=f"rot_{t}_{idx}")
                    nc.vector.tensor_tensor(
                        rot[:], rot_ps[:], st_t[:, tsl],
                        mybir.AluOpType.mult)
                    nc.vector.tensor_tensor(
                        x, x, ct[:, tsl], mybir.AluOpType.mult)
                    nc.vector.tensor_tensor(
                        x, x, rot[:], mybir.AluOpType.add)

                # rope/vt are emitted one c-group late so the PSUM->SBUF
                # copy they read never stalls the in-order PE stream.
                proj_col(4)                      # k
                proj_col(5)                      # v
                rope_col(QH)                     # k rope (c4 copy now done)
                proj_col(0)
                for j in range(TC // P):         # v transpose
                    kc = t * (TC // P) + j
                    tp = psp.tile([P, TC], bf16, tag="ps", name=f"vt_{kc}")
                    nc.tensor.transpose(
                        tp[:, 0:P], qkvT[:, 5, kc * P:(kc + 1) * P], idt[:])
                    nc.scalar.copy(vnat[:, kc, :], tp[:, 0:P])
                proj_col(1)
                rope_col(0)
                proj_col(2)
                rope_col(1)
                proj_col(3)
                rope_col(2)

                if t == 0:
                    # wo load, deferred so it never races the hot path
                    for hc in range(QH):
                        nc.sync.dma_start(wo_r[:, hc, :],
                                          wo_s[hc * P:(hc + 1) * P, :])

                # ---- phase 4: causal attention, q group g == t ----
                attnT = attp.tile([P, QH, TC], bf16, tag="attnT",
                                  name=f"attnT_{t}")
                kmax = 4 * (t + 1)
                for head in range(QH):
                    if head == 1:
                        rope_col(3)              # q3 rope hidden under head 0
                    d_rep = accp.tile([P, TC], f32, tag="d",
                                      name=f"d_{t}_{head}")
                    pv = accp.tile([P, TC], f32, tag="pv",
                                   name=f"pv_{t}_{head}")
                    es = []

                    def drain_one():
                        pkc, pe, plo = es.pop(0)
                        nc.tensor.matmul(d_rep[:, plo:], o_m[:],
                                         pe[:, plo:],
                                         start=(pkc == 0),
                                         stop=(pkc == kmax - 1))
                        nc.tensor.matmul(pv[:, plo:], vnat[:, pkc, :],
                                         pe[:, plo:],
                                         start=(pkc == 0),
                                         stop=(pkc == kmax - 1))

                    for kc in range(kmax):
                        j = kc - 4 * t
                        lo = max(j, 0) * P   # cols < lo are fully masked
                        st_ps = psp.tile([P, TC], f32, tag="ps",
                                         name=f"st_{t}_{head}_{kc}")
                        nc.tensor.matmul(
                            st_ps[:, lo:],
                            qkvT[:, QH, kc * P:(kc + 1) * P],
                            qkvT[:, head, t * TC + lo:(t + 1) * TC],
                            start=True, stop=True)
                        if len(es) >= LAG:
                            drain_one()
                        e = ep.tile([P, TC], bf16, tag="E",
                                    name=f"e_{t}_{head}_{kc}")
                        if j >= 0:
                            # staircase lives in cols [128j, 128j+128)
                            nc.vector.tensor_tensor(
                                st_ps[:, lo:lo + P],
                                st_ps[:, lo:lo + P], mt[:],
                                mybir.AluOpType.add)
                        nc.scalar.activation(
                            e[:, lo:], st_ps[:, lo:],
                            mybir.ActivationFunctionType.Exp,
                            scale=EXP_SCALE)
                        es.append((kc, e, lo))
                    while es:
                        drain_one()
                    rbs = rbsp.tile([P, TC], f32, tag="rbs",
                                    name=f"rbs_{t}_{head}")
                    nc.vector.reciprocal_approx_fast(rbs[:], d_rep[:])
                    nc.vector.tensor_tensor(
                        attnT[:, head, :], pv[:], rbs[:],
                        mybir.AluOpType.mult)

                # ---- phase 5: out chunk = attn(t) @ wo_shard ----
                for oc in range(NOC):
                    for tcn in range(TC // P):
                        o_ps = psp.tile([P, TC], f32, tag="ps",
                                         name=f"o_{t}_{oc}_{tcn}")
                        for hc in range(QH):
                            nc.tensor.matmul(
                                o_ps[:],
                                attnT[:, hc, tcn * P:(tcn + 1) * P],
                                wo_r[:, hc, oc * TC:(oc + 1) * TC],
                                start=(hc == 0), stop=(hc == QH - 1))
                        ob = obp.tile([P, TC], bf16, tag="ob",
                                      name=f"ob_{t}_{oc}_{tcn}")
                        if (oc + tcn) % 2 == 0:
                            nc.scalar.copy(ob[:], o_ps[:])
                        else:
                            nc.vector.tensor_copy(ob[:], o_ps[:])
                        nc.gpsimd.dma_start(
                            part[t * TC + tcn * P:t * TC + (tcn + 1) * P,
                                 oc * TC:(oc + 1) * TC], ob[:])

    nc.compile()
    return nc


def _rope_tables(positions):
    pos = positions.astype(np.float64)
    inv_freq = 1.0 / (BASE ** (np.arange(HALF, dtype=np.float64) / HALF))
    freqs = pos[:, None] * inv_freq[None, :]          # [T, 64]
    cos = np.cos(freqs)
    sin = np.sin(freqs)
    cosT = np.concatenate([cos, cos], axis=1).T       # [128, T]
    sinT = np.concatenate([-sin, sin], axis=1).T      # sign folded
    return cosT, sinT


def kernel(positions, hidden_states, wqkv, wo):
    global _COMPILED
    if _COMPILED is None:
        _COMPILED = _build()
    nc = _COMPILED

    s = 128.0 ** -0.5                                 # per-side score scale
    cosT, sinT = _rope_tables(positions)
    cosq = np.ascontiguousarray(cosT * s).astype(ml_dtypes.bfloat16)
    sinq = np.ascontiguousarray(sinT * s).astype(ml_dtypes.bfloat16)

    hidT = np.ascontiguousarray(hidden_states.T).astype(ml_dtypes.bfloat16)

    # causal staircase mask, ST layout [k, q]: one [128,128] tile serves
    # every diagonal block
    kl = np.arange(P)[:, None]
    ql = np.arange(P)[None, :]
    masks = np.where(kl <= ql, 0.0, NEG).astype(ml_dtypes.bfloat16)

    ident = np.eye(P, dtype=np.float32).astype(ml_dtypes.bfloat16)
    onesm = np.ones((P, P), dtype=np.float32).astype(ml_dtypes.bfloat16)
    rperm_np = np.zeros((P, P), dtype=np.float32)
    for m in range(P):
        rperm_np[(m + HALF) % P, m] = 1.0             # out[m]=x[(m+64)%128]
    rperm_np = rperm_np.astype(ml_dtypes.bfloat16)

    in_maps = []
    for r in range(NCORES):
        qc = slice(r * QCOLS, (r + 1) * QCOLS)
        kc = slice(NH * HD + r * HD, NH * HD + (r + 1) * HD)
        vc = slice((NH + NKV) * HD + r * HD, (NH + NKV) * HD + (r + 1) * HD)
        wqkv_s = np.ascontiguousarray(
            np.concatenate([wqkv[:, qc], wqkv[:, kc], wqkv[:, vc]],
                           axis=1)).astype(ml_dtypes.bfloat16)
        wo_s = np.ascontiguousarray(wo[qc, :]).astype(ml_dtypes.bfloat16)
        in_maps.append({
            "hidT": hidT, "wqkv_s": wqkv_s, "wo_s": wo_s,
            "cosq": cosq, "sinq": sinq, "masks": masks,
            "ident": ident, "onesm": onesm, "rperm": rperm_np,
        })

    global _LAST_IN_MAPS
    _LAST_IN_MAPS = in_maps
    res = run_bass_kernel_spmd(nc, in_maps, list(range(NCORES)))
    out = res.results[0]["part"].astype(np.float64)
    for r in range(1, NCORES):
        out += res.results[r]["part"].astype(np.float64)
    return out.astype(np.float32)


# revision 35
# speedup vs baseline: 1.0675x; 1.0008x over previous
"""InternLM3 self-attention (prefill, GQA, RoPE) on 8 Trainium2 cores.

Tensor-parallel over heads: core r owns q heads 4r..4r+3 and kv head r
(wqkv column shards, wo row shards).  Each core computes its partial
output projection; the 8 partials are summed on the host.

v3 design:
  - all matmuls bf16 (Fast Weight Load on; fp32r would serialize
    LDWEIGHTS at ~125 ns/matmul).
  - one software pipeline over the 4 token chunks: proj(t) -> rope(t)
    -> attention(g=t) -> out-proj(t); per-tile semaphores overlap the
    phases across engines.
  - softmax denominator accumulated on PE with an all-ones [128,128]
    stationary (result replicated across partitions), so 1/d is a single
    reciprocal_approx_fast on [128,512] and the normalizing multiply
    needs no broadcast at all.
  - scores/pv/denominator matmuls run 2 chunks behind the scores stream
    so PSUM bank recycling at head boundaries never stalls the PE.
  - RoPE rotate-half via SBUF->SBUF DMA partition shift; k head is
    roped first so attention never waits on it.
  - q and k both carry 128^-0.5 in the rope tables; exp() applies the
    compensating sqrt(128) via its free affine scale.
  - batched weight DMAs (descriptors fan out across all 16 queues);
    wo load is emitted late so it never delays the first projection.
"""

import numpy as np
import ml_dtypes

import concourse.bass as bass
import concourse.bacc as bacc
import concourse.mybir as mybir
import concourse.tile as tile
from concourse.bass_utils import run_bass_kernel_spmd

T = 2048
H = 4096
NH = 32
NKV = 8
HD = 128
HALF = HD // 2
BASE = 1000000.0
NCORES = 8
QH = NH // NCORES            # 4 q heads per core
QCOLS = QH * HD              # 512
SH_COLS = QCOLS + 2 * HD     # 768 wqkv cols per core
NEG = -1e30

P = 128
TC = 512                     # token chunk
NT = T // TC                 # 4
NHC = H // P                 # 32 contraction chunks
NQC = SH_COLS // P           # 6 qkv col chunks (0-3 q heads, 4 k, 5 v)
NOC = H // TC                # 8 output col chunks
LAG = 4                      # chunks the pv/denominator stream trails by

f32 = mybir.dt.float32
bf16 = mybir.dt.bfloat16

EXP_SCALE = float(np.sqrt(128.0))   # tables carry 128^-0.5 on q and k

_COMPILED = None


def _build():
    nc = bacc.Bacc("TRN2", target_bir_lowering=False, debug=False,
                   num_devices=NCORES)

    hidT = nc.dram_tensor("hidT", [H, T], bf16, kind="ExternalInput").ap()
    wqkv_s = nc.dram_tensor("wqkv_s", [H, SH_COLS], bf16,
                            kind="ExternalInput").ap()
    wo_s = nc.dram_tensor("wo_s", [QCOLS, H], bf16,
                          kind="ExternalInput").ap()
    cosq = nc.dram_tensor("cosq", [P, T], bf16, kind="ExternalInput").ap()
    sinq = nc.dram_tensor("sinq", [P, T], bf16, kind="ExternalInput").ap()
    masks = nc.dram_tensor("masks", [P, P], bf16,
                           kind="ExternalInput").ap()
    ident = nc.dram_tensor("ident", [P, P], bf16, kind="ExternalInput").ap()
    onesm = nc.dram_tensor("onesm", [P, P], bf16, kind="ExternalInput").ap()
    rperm = nc.dram_tensor("rperm", [P, P], bf16, kind="ExternalInput").ap()
    part = nc.dram_tensor("part", [T, H], bf16, kind="ExternalOutput").ap()

    with tile.TileContext(nc) as tc:
        with tc.tile_pool(name="keep", bufs=1) as keep, \
             tc.tile_pool(name="hid", bufs=10) as hidp, \
             tc.tile_pool(name="attn", bufs=2) as attp, \
             tc.tile_pool(name="rot", bufs=2) as rotp, \
             tc.tile_pool(name="e", bufs=7) as ep, \
             tc.tile_pool(name="rbs", bufs=2) as rbsp, \
             tc.tile_pool(name="ob", bufs=6) as obp, \
             tc.tile_pool(name="ps", bufs=6, space="PSUM") as psp, \
             tc.tile_pool(name="acc", bufs=1, space="PSUM") as accp:

            # ---- long-lived SBUF ----
            wq = keep.tile([P, NHC, SH_COLS], bf16)        # 48 KB/part
            wo_r = keep.tile([P, QH, H], bf16)             # 32 KB/part
            qkvT = keep.tile([P, NQC, T], bf16)            # 24 KB/part
            ct = keep.tile([P, T], bf16, tag="cosq_t")     # 4 KB
            st_t = keep.tile([P, T], bf16, tag="sinq_t")   # 4 KB
            mt = keep.tile([P, P], bf16, tag="masks_t")    # staircase
            vnat = keep.tile([P, T // P, P], bf16, tag="vnat")  # 4 KB
            idt = keep.tile([P, P], bf16, tag="ident_t")
            o_m = keep.tile([P, P], bf16, tag="onesm_t")
            rp = keep.tile([P, P], bf16, tag="rperm_t")

            # hid(0) + wq interleaved in first-needed order: the DMA queues
            # are FIFO, so issue order decides who gets early bandwidth.
            QTR = NHC // 8
            hid_t0 = [hidp.tile([P, QTR, TC], bf16, tag="hid",
                                name=f"hid_0_{qi}") for qi in range(8)]

            def hid_dma(hq, qi, t):
                nc.sync.dma_start(
                    hq[:],
                    hidT[qi * QTR * P:(qi + 1) * QTR * P,
                         t * TC:(t + 1) * TC].rearrange(
                        "(h p) n -> p h n", p=P))

            def wq_dma(c, half):
                hh = half * (NHC // 2)
                nc.sync.dma_start(
                    wq[:, hh:hh + NHC // 2, c * P:(c + 1) * P],
                    wqkv_s[hh * P:hh * P + NHC // 2 * P,
                           c * P:(c + 1) * P].rearrange(
                        "(h p) c -> p h c", p=P))

            def wq_dma_q(c, q8):
                hh = q8 * (NHC // 4)
                nc.sync.dma_start(
                    wq[:, hh:hh + NHC // 4, c * P:(c + 1) * P],
                    wqkv_s[hh * P:hh * P + NHC // 4 * P,
                           c * P:(c + 1) * P].rearrange(
                        "(h p) c -> p h c", p=P))

            hid_dma(hid_t0[0], 0, 0)
            wq_dma_q(4, 0)
            hid_dma(hid_t0[1], 1, 0)
            wq_dma_q(4, 1)
            hid_dma(hid_t0[2], 2, 0)
            wq_dma_q(4, 2)
            hid_dma(hid_t0[3], 3, 0)
            wq_dma_q(4, 3)
            for qi in range(4, 8):
                hid_dma(hid_t0[qi], qi, 0)
            for c in (5, 0, 1, 2, 3):
                wq_dma(c, 0)
                wq_dma(c, 1)
            nc.scalar.dma_start(ct[:], cosq[:])
            nc.scalar.dma_start(st_t[:], sinq[:])
            nc.scalar.dma_start(mt[:], masks[:])
            nc.scalar.dma_start(idt[:], ident[:])
            nc.scalar.dma_start(o_m[:], onesm[:])
            nc.scalar.dma_start(rp[:], rperm[:])

            for t in range(NT):
                tsl = slice(t * TC, (t + 1) * TC)

                if t == 0:
                    hid_q = hid_t0
                else:
                    hid_q = []
                    for qi in range(8):
                        hq = hidp.tile([P, QTR, TC], bf16, tag="hid",
                                       name=f"hid_{t}_{qi}")
                        hid_dma(hq, qi, t)
                        hid_q.append(hq)

                # ---- phase 1: qkv^T chunk = wqkv^T @ hidden^T ----
                # k and v first so rope(k) / v-transpose overlap the q cols.
                def proj_col(c):
                    qps = psp.tile([P, TC], f32, tag="ps",
                                   name=f"qps_{t}_{c}")
                    for h in range(NHC):
                        nc.tensor.matmul(
                            qps[:], wq[:, h, c * P:(c + 1) * P],
                            hid_q[h // QTR][:, h % QTR, :],
                            start=(h == 0), stop=(h == NHC - 1))
                    nc.scalar.copy(qkvT[:, c, tsl], qps[:])

                def rope_col(idx):
                    x = qkvT[:, idx, tsl]
                    rot_ps = psp.tile([P, TC], f32, tag="ps",
                                      name=f"rotp_{t}_{idx}")
                    nc.tensor.matmul(rot_ps[:], rp[:], x,
                                     start=True, stop=True)
                    rot = rotp.tile([P, TC], bf16, tag="rot",
                                    name=f"rot_{t}_{idx}")
                    nc.vector.tensor_tensor(
                        rot[:], rot_ps[:], st_t[:, tsl],
                        mybir.AluOpType.mult)
                    nc.vector.tensor_tensor(
                        x, x, ct[:, tsl], mybir.AluOpType.mult)
                    nc.vector.tensor_tensor(
                        x, x, rot[:], mybir.AluOpType.add)

                # rope/vt are emitted one c-group late so the PSUM->SBUF
                # copy they read never stalls the in-order PE stream.
                proj_col(4)                      # k
                proj_col(5)                      # v
                rope_col(QH)                     # k rope (c4 copy now done)
                proj_col(0)
                for j in range(TC // P):         # v transpose
                    kc = t * (TC // P) + j
                    tp = psp.tile([P, TC], bf16, tag="ps", name=f"vt_{kc}")
                    nc.tensor.transpose(
                        tp[:, 0:P], qkvT[:, 5, kc * P:(kc + 1) * P], idt[:])
                    nc.scalar.copy(vnat[:, kc, :], tp[:, 0:P])
                proj_col(1)
                rope_col(0)
                proj_col(2)
                rope_col(1)
                proj_col(3)
                rope_col(2)

                if t == 0:
                    # wo load, deferred so it never races the hot path
                    for hc in range(QH):
                        nc.sync.dma_start(wo_r[:, hc, :],
                                          wo_s[hc * P:(hc + 1) * P, :])

                # ---- phase 4: causal attention, q group g == t ----
                attnT = attp.tile([P, QH, TC], bf16, tag="attnT",
                                  name=f"attnT_{t}")
                kmax = 4 * (t + 1)
                for head in range(QH):
                    if head == 1:
                        rope_col(3)              # q3 rope hidden under head 0
                    d_rep = accp.tile([P, TC], f32, tag="d",
                                      name=f"d_{t}_{head}")
                    pv = accp.tile([P, TC], f32, tag="pv",
                                   name=f"pv_{t}_{head}")
                    es = []

                    def drain_one():
                        pkc, pe, plo = es.pop(0)
                        nc.tensor.matmul(d_rep[:, plo:], o_m[:],
                                         pe[:, plo:],
                                         start=(pkc == 0),
                                         stop=(pkc == kmax - 1))
                        nc.tensor.matmul(pv[:, plo:], vnat[:, pkc, :],
                                         pe[:, plo:],
                                         start=(pkc == 0),
                                         stop=(pkc == kmax - 1))

                    for kc in range(kmax):
                        j = kc - 4 * t
                        lo = max(j, 0) * P   # cols < lo are fully masked
                        st_ps = psp.tile([P, TC], f32, tag="ps",
                                         name=f"st_{t}_{head}_{kc}")
                        nc.tensor.matmul(
                            st_ps[:, lo:],
                            qkvT[:, QH, kc * P:(kc + 1) * P],
                            qkvT[:, head, t * TC + lo:(t + 1) * TC],
                            start=True, stop=True)
                        if len(es) >= LAG:
                            drain_one()
                        e = ep.tile([P, TC], bf16, tag="E",
                                    name=f"e_{t}_{head}_{kc}")
                        if j >= 0:
                            # staircase lives in cols [128j, 128j+128)
                            nc.vector.tensor_tensor(
                                st_ps[:, lo:lo + P],
                                st_ps[:, lo:lo + P], mt[:],
                                mybir.AluOpType.add)
                        nc.scalar.activation(
                            e[:, lo:], st_ps[:, lo:],
                            mybir.ActivationFunctionType.Exp,
                            scale=EXP_SCALE)
                        es.append((kc, e, lo))
                    while es:
                        drain_one()
                    rbs = rbsp.tile([P, TC], f32, tag="rbs",
                                    name=f"rbs_{t}_{head}")
                    nc.vector.reciprocal_approx_fast(rbs[:], d_rep[:])
                    nc.vector.tensor_tensor(
                        attnT[:, head, :], pv[:], rbs[:],
                        mybir.AluOpType.mult)

                # ---- phase 5: out chunk = attn(t) @ wo_shard ----
                for oc in range(NOC):
                    for tcn in range(TC // P):
                        o_ps = psp.tile([P, TC], f32, tag="ps",
                                         name=f"o_{t}_{oc}_{tcn}")
                        for hc in range(QH):
                            nc.tensor.matmul(
                                o_ps[:],
                                attnT[:, hc, tcn * P:(tcn + 1) * P],
                                wo_r[:, hc, oc * TC:(oc + 1) * TC],
                                start=(hc == 0), stop=(hc == QH - 1))
                        ob = obp.tile([P, TC], bf16, tag="ob",
                                      name=f"ob_{t}_{oc}_{tcn}")
                        if (oc + tcn) % 2 == 0:
                            nc.scalar.copy(ob[:], o_ps[:])
                        else:
                            nc.vector.tensor_copy(ob[:], o_ps[:])
                        nc.gpsimd.dma_start(
                            part[t * TC + tcn * P:t * TC + (tcn + 1) * P,
                                 oc * TC:(oc + 1) * TC], ob[:])

    nc.compile()
    return nc


def _rope_tables(positions):
    pos = positions.astype(np.float64)
    inv_freq = 1.0 / (BASE ** (np.arange(HALF, dtype=np.float64) / HALF))
    freqs = pos[:, None] * inv_freq[None, :]          # [T, 64]
    cos = np.cos(freqs)
    sin = np.sin(freqs)
    cosT = np.concatenate([cos, cos], axis=1).T       # [128, T]
    sinT = np.concatenate([-sin, sin], axis=1).T      # sign folded
    return cosT, sinT


def kernel(positions, hidden_states, wqkv, wo):
    global _COMPILED
    if _COMPILED is None:
        _COMPILED = _build()
    nc = _COMPILED

    s = 128.0 ** -0.5                                 # per-side score scale
    cosT, sinT = _rope_tables(positions)
    cosq = np.ascontiguousarray(cosT * s).astype(ml_dtypes.bfloat16)
    sinq = np.ascontiguousarray(sinT * s).astype(ml_dtypes.bfloat16)

    hidT = np.ascontiguousarray(hidden_states.T).astype(ml_dtypes.bfloat16)

    # causal staircase mask, ST layout [k, q]: one [128,128] tile serves
    # every diagonal block
    kl = np.arange(P)[:, None]
    ql = np.arange(P)[None, :]
    masks = np.where(kl <= ql, 0.0, NEG).astype(ml_dtypes.bfloat16)

    ident = np.eye(P, dtype=np.float32).astype(ml_dtypes.bfloat16)
    onesm = np.ones((P, P), dtype=np.float32).astype(ml_dtypes.bfloat16)
    rperm_np = np.zeros((P, P), dtype=np.float32)
    for m in range(P):
        rperm_np[(m + HALF) % P, m] = 1.0             # out[m]=x[(m+64)%128]
    rperm_np = rperm_np.astype(ml_dtypes.bfloat16)

    in_maps = []
    for r in range(NCORES):
        qc = slice(r * QCOLS, (r + 1) * QCOLS)
        kc = slice(NH * HD + r * HD, NH * HD + (r + 1) * HD)
        vc = slice((NH + NKV) * HD + r * HD, (NH + NKV) * HD + (r + 1) * HD)
        wqkv_s = np.ascontiguousarray(
            np.concatenate([wqkv[:, qc], wqkv[:, kc], wqkv[:, vc]],
                           axis=1)).astype(ml_dtypes.bfloat16)
        wo_s = np.ascontiguousarray(wo[qc, :]).astype(ml_dtypes.bfloat16)
        in_maps.append({
            "hidT": hidT, "wqkv_s": wqkv_s, "wo_s": wo_s,
            "cosq": cosq, "sinq": sinq, "masks": masks,
            "ident": ident, "onesm": onesm, "rperm": rperm_np,
        })

    global _LAST_IN_MAPS
    _LAST_IN_MAPS = in_maps
    res = run_bass_kernel_spmd(nc, in_maps, list(range(NCORES)))
    out = res.results[0]["part"].astype(np.float64)
    for r in range(1, NCORES):
        out += res.results[r]["part"].astype(np.float64)
    return out.astype(np.float32)
